# revision 1
# baseline (speedup 1.0000x reference)
"""Contour-to-mask winding-number kernel for 8 Trainium2 NeuronCores.

Problem: for each of 16 contours (64 vertices each) and each pixel of a
128x128 grid, sum over polygon edges k:
    tanh(1e5*cross_k) * acos(clip(dot_k / (|d_k||rd_k|), -1+eps, 1-eps))
then |sum| / 2pi clipped to [0, 1].

Math used on device (validated offline vs the jax reference on the exact
setup_inputs() instance: L2 rel 6.3e-4, max abs 4.8e-3, tolerance 2e-2):

  cross(p,i,j) = (cy-y)(rx-x) - (cx-x)(ry-y)  -- the xy terms cancel ->
               = CXI(p,i) + CYJ(p,j)          (separable sum!)
  dot(p,i,j)   = DXI(p,i) + DYJ(p,j)          (separable sum)
  with t = tanh(1e5*cross), g = atan(dot * recip(cross)):
  summand = t * atan2(|cross|, dot) = t*(pi/2 - sign(cross)*g)
          = (pi/2)*t - |t|*g
  so the edge sum is two accumulating TensorE matmuls (masks 1/2pi and
  -1/4; global sign flip absorbed by the final |sum|), and the
  full-size elementwise ops are: 2 broadcast adds, recip (ScalarE),
  q=dot*r, Arctan, Tanh, |t| and |t|*g  (the last two in f16).

CXI/CYJ/DXI/DYJ are tiny [128, 128] per-edge tiles precomputed on host.
Layout per core (2 contours): SBUF partition p = contour*64 + edge k,
free dim = pixel (i major, j minor). Full-size [128, 16384] arrays built
via free-dim stride-0 broadcast adds of the tiny tiles. g/t/w computed in
f16 (validated); edge sum via f16 TensorE matmul against a 0/1 mask
scaled by 1/2pi, contracting 128 partitions to the 2 contours.
"""

import math

import numpy as np

B, N, KV, S = 2, 8, 64, 128
S2 = S * S
NCON = B * N
NCORES = 8
CPC = NCON // NCORES  # contours per core

CHUNK = 1024  # pixels per full-size tile
NCHUNK = S2 // CHUNK
IBLK = CHUNK // S  # i values per chunk

PI = float(np.float32(math.pi))
INV2PI = float(np.float32(1.0 / (2.0 * math.pi)))
K_SIGN = 1.0e5

def _act_raw(nc, out, in_, func, bias=0.0, scale=1.0, alpha=0.0):
    """Emit InstActivation directly.  Needed for Reciprocal, which bass's
    Python wrapper refuses; measured accuracy ~1.2e-5 max rel, ample here."""
    import concourse.mybir as mybir

    se = nc.scalar
    ins = [se.lower_ap(in_)]
    for arg in (bias, scale, alpha):
        ins.append(mybir.ImmediateValue(dtype=mybir.dt.float32, value=float(arg)))
    return se.add_instruction(
        mybir.InstActivation(
            name=nc.get_next_instruction_name(),
            func=func,
            ins=ins,
            outs=[se.lower_ap(out)],
        )
    )



_CACHE = {}


# --------------------------------------------------------------------------
# workaround: walrus rejects instructions carrying many sem waits; Tile's
# exit drain waits on every used semaphore.  Split across several drains.
def _patch_tile_drain():
    import bass_rust
    import concourse.tile as tile

    if getattr(tile.TileContext, "_ctm_drain_patched", False):
        return
    MAX_WAITS = 1

    def _drain_and_barrier(self, tick_clock, wait_clock):
        from concourse.vector_clock import ScopedClock

        nc = self.nc
        drain_inst = nc.sync.drain()
        wait_clock.add_sem_waits(
            drain_inst.ins, ScopedClock({None: tick_clock.global_clock})
        )
        si = drain_inst.ins.sync_info
        if si is not None and len(si.on_wait) > MAX_WAITS:
            waits = list(si.on_wait)
            drain_inst.ins.sync_info = bass_rust.SyncInfo(
                on_wait=waits[:MAX_WAITS], on_update=list(si.on_update)
            )
            for off in range(MAX_WAITS, len(waits), MAX_WAITS):
                extra = nc.sync.drain()
                extra.ins.sync_info = bass_rust.SyncInfo(
                    on_wait=waits[off : off + MAX_WAITS], on_update=[]
                )
        nc.all_engine_barrier()
        popped = nc._tile_sem_poison_stack.pop()
        assert popped is self._sem_poison
        nc.clear_and_free_semaphores(list(self.sems.allocated().values()))
        nc.all_engine_barrier()

    tile.TileContext._drain_and_barrier = _drain_and_barrier
    tile.TileContext._ctm_drain_patched = True


def _split_sync_waits(nc, max_waits=1):
    """Walrus codegen rejects instructions carrying more than a couple of sem
    waits.  Move excess waits onto same-engine NOPs inserted just before."""
    import bass_rust

    n = 0
    for fn in nc.m.functions:
        for blk in fn.blocks:
            insts = blk.instructions
            out = []
            for inst in insts:
                si = inst.sync_info
                waits = list(si.on_wait) if si is not None else []
                if len(waits) > max_waits:
                    for off in range(max_waits, len(waits), max_waits):
                        nop = bass_rust.InstNoOp(name=f"ctm_waitnop_{n}", ins=[], outs=[])
                        n += 1
                        nop.engine = inst.engine
                        nop.sync_info = bass_rust.SyncInfo(
                            on_wait=waits[off : off + max_waits], on_update=[]
                        )
                        out.append(nop)
                    inst.sync_info = bass_rust.SyncInfo(
                        on_wait=waits[:max_waits], on_update=list(si.on_update)
                    )
                out.append(inst)
            if n:
                blk.instructions = out
    return n


# --------------------------------------------------------------------------
def _build_bass(repeat=1):
    """Build the per-core Bass module (identical on all 8 cores).

    repeat>1 re-runs the whole compute that many times (same tiles) --
    used only for slope-based HW timing in test.py."""
    from contextlib import ExitStack

    import concourse.bass as bass
    import concourse.mybir as mybir
    import concourse.tile as tile

    _patch_tile_drain()
    F32 = mybir.dt.float32
    F16 = mybir.dt.float16
    AF = mybir.ActivationFunctionType
    Alu = mybir.AluOpType

    nc = bass.Bass()
    coef = nc.dram_tensor("coef", [128, 4 * S], F32, kind="ExternalInput")
    maskw = nc.dram_tensor("maskw", [128, 2 * CPC], F16, kind="ExternalInput")
    out = nc.dram_tensor("out", [CPC, S2], F32, kind="ExternalOutput")

    with tile.TileContext(nc) as tc, ExitStack() as ctx:
        const = ctx.enter_context(tc.tile_pool(name="const", bufs=1))
        pa = ctx.enter_context(tc.tile_pool(name="pa", bufs=4))
        pc = ctx.enter_context(tc.tile_pool(name="pc", bufs=4))
        pq = ctx.enter_context(tc.tile_pool(name="pq", bufs=4))
        pg = ctx.enter_context(tc.tile_pool(name="pg", bufs=4))
        pt = ctx.enter_context(tc.tile_pool(name="pt", bufs=4))
        pw = ctx.enter_context(tc.tile_pool(name="pw", bufs=4))
        pn = ctx.enter_context(tc.tile_pool(name="pn", bufs=4))
        pz = ctx.enter_context(tc.tile_pool(name="pz", bufs=4))
        pr = ctx.enter_context(tc.tile_pool(name="pr", bufs=4))
        psum = ctx.enter_context(tc.tile_pool(name="ps", bufs=4, space="PSUM"))

        cf = const.tile([128, 4 * S], F32)
        nc.sync.dma_start(cf[:, : 2 * S], coef[:, : 2 * S])  # cxi/cyj first: 'a' can start
        mw = const.tile([128, 2 * CPC], F16)
        nc.sync.dma_start(mw[:], maskw[:])
        nc.sync.dma_start(cf[:, 2 * S :], coef[:, 2 * S :])
        mwu = mw[:, :CPC]        # 1/2pi mask for |t|*g
        mwt = mw[:, CPC:]        # -1/4 mask for t
        cxi = cf[:, 0 * S : 1 * S]
        cyj = cf[:, 1 * S : 2 * S]
        dxi = cf[:, 2 * S : 3 * S]
        dyj = cf[:, 3 * S : 4 * S]

        final = const.tile([128, CPC * S], F32)

        for ch in range(NCHUNK * repeat):
            ch = ch % NCHUNK
            i0 = ch * IBLK
            sh3 = [128, IBLK, S]

            def bj(t_):  # broadcast a [128, S] j-tile over the i axis
                return t_.unsqueeze(1).broadcast_to(sh3)

            def bi(t_):  # broadcast this chunk's i-slice over the j axis
                return t_[:, i0 : i0 + IBLK].unsqueeze(2).broadcast_to(sh3)

            a = pa.tile([128, CHUNK], F32)
            c = pc.tile([128, CHUNK], F32)
            rr = pn.tile([128, CHUNK], F32)
            q = pq.tile([128, CHUNK], F32)
            g = pg.tile([128, CHUNK], F16)
            t = pt.tile([128, CHUNK], F16)
            ta = pz.tile([128, CHUNK], F16)
            w = pw.tile([128, CHUNK], F16)
            a3 = a[:].rearrange("p (x y) -> p x y", x=IBLK)
            c3 = c[:].rearrange("p (x y) -> p x y", x=IBLK)

            # a = cross, c = dot (both separable sums of tiny tiles)
            nc.vector.tensor_tensor(out=a3, in0=bi(cxi), in1=bj(cyj), op=Alu.add)
            nc.gpsimd.tensor_tensor(out=c3, in0=bi(dxi), in1=bj(dyj), op=Alu.add)

            # q = dot * recip(cross)  (f32, signed; finite since cross != 0)
            _act_raw(nc, rr[:], a[:], AF.Reciprocal)
            nc.vector.tensor_tensor(out=q[:], in0=c[:], in1=rr[:], op=Alu.mult)

            # g = atan(q), t = tanh(1e5*cross), ta = |t|, w = |t|*g  (f16)
            nc.scalar.activation(g[:], q[:], AF.Arctan)
            nc.scalar.activation(t[:], a[:], AF.Tanh, 0.0, K_SIGN)
            nc.vector.scalar_tensor_tensor(out=ta[:], in0=t[:], scalar=-1.0,
                                           in1=t[:], op0=Alu.mult, op1=Alu.max)
            nc.vector.tensor_tensor(out=w[:], in0=ta[:], in1=g[:], op=Alu.mult)

            # edge-sum via PE: accumulate (1/2pi)*w + (-1/4)*t per 512 block
            ps = psum.tile([CPC, CHUNK], F32)
            for m in range(CHUNK // 512):
                blk = slice(m * 512, (m + 1) * 512)
                nc.tensor.matmul(ps[:, blk], mwu, w[:, blk], start=True, stop=False)
                nc.tensor.matmul(ps[:, blk], mwt, t[:, blk], start=False, stop=True)
            red = pr.tile([CPC, CHUNK], F32)
            nc.scalar.activation(red[:], ps[:], AF.Abs)

            # redistribute onto final's [i, contour*S + j] layout as we go
            for cc in range(CPC):
                nc.sync.dma_start(final[i0 : i0 + IBLK, cc * S : (cc + 1) * S],
                                  red[cc : cc + 1, :])

        nc.vector.tensor_scalar(out=final[:], in0=final[:], scalar1=1.0,
                                scalar2=None, op0=Alu.min)
        nc.sync.dma_start(out[:].rearrange("c (i j) -> i c j", i=S), final[:])

    _split_sync_waits(nc)
    return nc


def _get_nc():
    if "nc" not in _CACHE:
        _CACHE["nc"] = _build_bass()
    return _CACHE["nc"]


def _make_in_maps(contour):
    c = contour.reshape(NCON, KV, 2)
    cx, cy = c[:, :, 0], c[:, :, 1]
    rx, ry = np.roll(cx, -1, 1), np.roll(cy, -1, 1)
    x = np.arange(S, dtype=np.float32) / np.float32(S)
    # cross = CXI(p,i) + CYJ(p,j); dot = DXI(p,i) + DYJ(p,j)
    CXI = (cy * rx - cx * ry)[:, :, None] + (ry - cy)[:, :, None] * x
    CYJ = (cx - rx)[:, :, None] * x + np.float32(0.0)
    DXI = x * x - (cx + rx)[:, :, None] * x + (cx * rx)[:, :, None]
    DYJ = x * x - (cy + ry)[:, :, None] * x + (cy * ry)[:, :, None]
    mask = np.zeros((128, 2 * CPC), np.float16)
    for lc in range(CPC):
        mask[lc * KV : (lc + 1) * KV, lc] = np.float16(INV2PI)
        mask[lc * KV : (lc + 1) * KV, CPC + lc] = np.float16(-0.25)
    in_maps = []
    for core in range(NCORES):
        coef = np.zeros((128, 4 * S), np.float32)
        for lc in range(CPC):
            p = core * CPC + lc
            rows = slice(lc * KV, (lc + 1) * KV)
            coef[rows, 0 * S : 1 * S] = CXI[p]
            coef[rows, 1 * S : 2 * S] = CYJ[p]
            coef[rows, 2 * S : 3 * S] = DXI[p]
            coef[rows, 3 * S : 4 * S] = DYJ[p]
        in_maps.append({"coef": coef, "maskw": mask})
    return in_maps


def kernel(contour, size):
    contour = np.asarray(contour, dtype=np.float32)
    size = int(size)
    assert contour.shape == (B, N, KV, 2), contour.shape
    assert size == S, size

    from concourse.bass_utils import run_bass_kernel_spmd

    nc = _get_nc()
    in_maps = _make_in_maps(contour)
    res = run_bass_kernel_spmd(nc, in_maps, core_ids=list(range(NCORES)))
    full = np.concatenate([res.results[i]["out"] for i in range(NCORES)], axis=0)
    return full.reshape(B, N, S, S).astype(np.float32)



# revision 18
# speedup vs baseline: 66.3053x; 66.3053x over previous
"""Contour-to-mask winding-number kernel for 8 Trainium2 NeuronCores.

Problem: for each of 16 contours (64 vertices each) and each pixel of a
128x128 grid, sum over polygon edges k:
    tanh(1e5*cross_k) * acos(clip(dot_k / (|d_k||rd_k|), -1+eps, 1-eps))
then |sum| / 2pi clipped to [0, 1].

Math used on device (validated offline vs the jax reference on the exact
setup_inputs() instance: L2 rel 2.4e-4, max abs 1.5e-2, tolerance 2e-2):

The per-edge summand sign(cross)*atan2(|cross|, dot) is the wrapped
angle delta wrap(alpha_{k+1} - alpha_k) of the pixel->vertex direction
angles; summed over a closed polygon the principal parts telescope to
zero and only branch-cut crossings survive.  The reference's
tanh-smoothed sum is therefore exactly a smoothed signed crossing
count of the horizontal scanline ray:

    sum_k t_k*theta_k / 2pi  ==  sum_k H_k(j) * (1 + tanh(g_k*(x_i - XC_k(j)))) / 2

where, per edge k:  H = +-1 if the edge crosses the line y=y_j (sign by
direction) else 0;  XC(j) = crossing x of the edge with y=y_j;
g = 1e5*|ry-cy| (cross = (ry-cy)*(x_i-XC) identically).  Since
sum_k H_k = 0 for a closed polygon, the (1+..)/2 constant drops and the
whole kernel is:

    u = SXI(p,i) - SXC(p,j)     (broadcast subtract of host tiles)
    T = tanh(u)                  (f16; the ONLY ScalarE op -> one act table)
    P = T_blk^T @ mask           (PE: T 128x128 block STATIONARY, mask
                                  [128,2] moving -> psum [128 j, 2])
    out = min(|P - C|, 1)        (two tiny DVE ops per half at the end)

H's sign equals sign(ry-cy) wherever it is nonzero, so the H multiply
folds into the tiles: SXI(p,i) = sg_p*x_i, SXC(p,j) = sg_p*XC_p(j) with
SIGNED sg = 1e5*(ry-cy), making tanh(u) = H*|tanh| for live cells.
Dead cells (edge does not cross scanline j) get SXC = min(0,sg)-50 so
u>=50 and T=+1.0 exactly; the host-known bias C(c,j) = 0.5*#dead is
subtracted at the end.  All tiny [128, 128] tiles precomputed on host.
Layout per core (2 contours): SBUF partition p = contour*64 + edge k,
free dim = pixel (i major, j minor).  Keeping the mask as the MOVING
matmul operand puts pixels on the PSUM partition dim, so the whole
per-core result [128 j, 256 (i,c)] fits in half a PSUM bank: no
per-chunk PSUM drain, and the host undoes the [j, (i,c)] layout.
"""

import math

import numpy as np

B, N, KV, S = 2, 8, 64, 128
S2 = S * S
NCON = B * N
NCORES = 8
CPC = NCON // NCORES  # contours per core

CHUNK = 4096  # pixels per full-size tile (max chunk size)

K_SIGN = 1.0e5

_CACHE = {}


# --------------------------------------------------------------------------
# workaround: walrus rejects instructions carrying many sem waits; Tile's
# exit drain waits on every used semaphore.  Split across several drains.
def _patch_tile_drain():
    import bass_rust
    import concourse.tile as tile

    if getattr(tile.TileContext, "_ctm_drain_patched", False):
        return
    MAX_WAITS = 1

    def _drain_and_barrier(self, tick_clock, wait_clock):
        from concourse.vector_clock import ScopedClock

        nc = self.nc
        drain_inst = nc.sync.drain()
        wait_clock.add_sem_waits(
            drain_inst.ins, ScopedClock({None: tick_clock.global_clock})
        )
        si = drain_inst.ins.sync_info
        if si is not None and len(si.on_wait) > MAX_WAITS:
            waits = list(si.on_wait)
            drain_inst.ins.sync_info = bass_rust.SyncInfo(
                on_wait=waits[:MAX_WAITS], on_update=list(si.on_update)
            )
            for off in range(MAX_WAITS, len(waits), MAX_WAITS):
                extra = nc.sync.drain()
                extra.ins.sync_info = bass_rust.SyncInfo(
                    on_wait=waits[off : off + MAX_WAITS], on_update=[]
                )
        nc.all_engine_barrier()
        popped = nc._tile_sem_poison_stack.pop()
        assert popped is self._sem_poison
        nc.clear_and_free_semaphores(list(self.sems.allocated().values()))
        nc.all_engine_barrier()

    tile.TileContext._drain_and_barrier = _drain_and_barrier
    tile.TileContext._ctm_drain_patched = True


def _split_sync_waits(nc, max_waits=1):
    """Walrus codegen rejects instructions carrying more than a couple of sem
    waits.  Move excess waits onto same-engine NOPs inserted just before."""
    import bass_rust

    n = 0
    for fn in nc.m.functions:
        for blk in fn.blocks:
            insts = blk.instructions
            out = []
            for inst in insts:
                si = inst.sync_info
                waits = list(si.on_wait) if si is not None else []
                if len(waits) > max_waits:
                    for off in range(max_waits, len(waits), max_waits):
                        nop = bass_rust.InstNoOp(name=f"ctm_waitnop_{n}", ins=[], outs=[])
                        n += 1
                        nop.engine = inst.engine
                        nop.sync_info = bass_rust.SyncInfo(
                            on_wait=waits[off : off + max_waits], on_update=[]
                        )
                        out.append(nop)
                    inst.sync_info = bass_rust.SyncInfo(
                        on_wait=waits[:max_waits], on_update=list(si.on_update)
                    )
                out.append(inst)
            if n:
                blk.instructions = out
    return n


# --------------------------------------------------------------------------
def _build_bass(repeat=1):
    """Build the per-core Bass module (identical on all 8 cores).

    repeat>1 re-runs the whole compute that many times (same tiles) --
    used only for slope-based HW timing in test.py."""
    from contextlib import ExitStack

    import concourse.bass as bass
    import concourse.mybir as mybir
    import concourse.tile as tile

    _patch_tile_drain()
    F32 = mybir.dt.float32
    F16 = mybir.dt.float16
    AF = mybir.ActivationFunctionType
    Alu = mybir.AluOpType

    nc = bass.Bass()
    cf32 = nc.dram_tensor("cf32", [128, 2 * S + CPC], F32, kind="ExternalInput")
    cf16 = nc.dram_tensor("cf16", [128, CPC], F16, kind="ExternalInput")
    out = nc.dram_tensor("out", [S, S * CPC], F32, kind="ExternalOutput")

    # (i0, iblk) per chunk: small first chunks start the tanh stream early,
    # large late chunks amortize per-instruction overhead
    CHUNKS = [(0, 8), (8, 8), (16, 16), (32, 32), (64, 32), (96, 32)]
    POOL_CHUNKS = {0, 1, 3}  # subtract runs on gpsimd for these chunks

    with tile.TileContext(nc) as tc, ExitStack() as ctx:
        const = ctx.enter_context(tc.tile_pool(name="const", bufs=1))
        pu = ctx.enter_context(tc.tile_pool(name="pu", bufs=4))
        pt = ctx.enter_context(tc.tile_pool(name="pt", bufs=3))
        psum = ctx.enter_context(tc.tile_pool(name="ps", bufs=1, space="PSUM"))

        # preload the tanh activation table while the input DMAs run
        scr = const.tile([1, 2], F32)
        nc.gpsimd.memset(scr[:], 0.0)
        nc.scalar.activation(scr[:, 1:], scr[:, :1], AF.Tanh)

        c32 = const.tile([128, 2 * S + CPC], F32)
        nc.sync.dma_start(c32[:], cf32[:])
        c16 = const.tile([128, CPC], F16)
        nc.sync.dma_start(c16[:], cf16[:])
        sxi = c32[:, :S]
        sxc = c32[:, S : 2 * S]
        w0 = c32[:, 2 * S :]  # dead-cell bias C per [j, contour]
        mw = c16[:]           # 0.5 contraction mask [edge-part, contour]

        # whole per-core result, pixels on partitions: [j, (i, contour)]
        ps = psum.tile([S, S * CPC], F32)
        final = const.tile([S, S * CPC], F32)

        def emit_half(half):
            # out = min(|P - C|, 1), drained per half so DMA overlaps compute
            lo, hi = half * S // 2 * CPC, (half + 1) * S // 2 * CPC
            sh = [128, S // 2, CPC]
            nc.vector.tensor_tensor(
                out=final[:, lo:hi].rearrange("p (x c) -> p x c", c=CPC),
                in0=ps[:, lo:hi].rearrange("p (x c) -> p x c", c=CPC),
                in1=w0.unsqueeze(1).broadcast_to(sh), op=Alu.subtract)
            nc.vector.scalar_tensor_tensor(
                out=final[:, lo:hi], in0=final[:, lo:hi], scalar=-1.0,
                in1=final[:, lo:hi], op0=Alu.mult, op1=Alu.max)
            nc.vector.tensor_scalar(out=final[:, lo:hi], in0=final[:, lo:hi],
                                    scalar1=1.0, scalar2=None, op0=Alu.min)
            nc.sync.dma_start(out[:, lo:hi], final[:, lo:hi])

        for ci in range(len(CHUNKS) * repeat):
            rep, ci = divmod(ci, len(CHUNKS))
            i0, iblk = CHUNKS[ci]
            sh3 = [128, iblk, S]

            def bj(t_):  # broadcast a [128, S] j-tile over the i axis
                return t_.unsqueeze(1).broadcast_to(sh3)

            def bi(t_):  # broadcast this chunk's i-slice over the j axis
                return t_[:, i0 : i0 + iblk].unsqueeze(2).broadcast_to(sh3)

            u = pu.tile([128, CHUNK], F32)
            t = pt.tile([128, CHUNK], F16)
            u3 = u[:, : iblk * S].rearrange("p (x y) -> p x y", x=iblk)

            # u = sg*(x_i - XC)  (separable);  T = tanh(u) = H*|tanh|
            eng = nc.gpsimd if ci in POOL_CHUNKS else nc.vector
            eng.tensor_tensor(out=u3, in0=bi(sxi), in1=bj(sxc), op=Alu.subtract)
            nc.scalar.activation(t[:, : iblk * S], u[:, : iblk * S], AF.Tanh)

            # edge-sum via PE: T i-row block [128, S] stationary, 0.5-mask
            # [128, CPC] moving -> psum[j, ((i0+b)*CPC) : +CPC]
            for b in range(iblk):
                gi = i0 + b
                nc.tensor.matmul(
                    ps[:, gi * CPC : (gi + 1) * CPC],
                    t[:, b * S : (b + 1) * S],
                    mw,
                    start=True,
                    stop=True,
                )
            if rep == repeat - 1 and i0 + iblk == S // 2:
                emit_half(0)
        emit_half(1)

    _split_sync_waits(nc)
    return nc


def _get_nc():
    if "nc" not in _CACHE:
        _CACHE["nc"] = _build_bass()
    return _CACHE["nc"]


def _make_in_maps(contour):
    c = contour.reshape(NCON, KV, 2).astype(np.float64)
    cx, cy = c[:, :, 0], c[:, :, 1]
    rx, ry = np.roll(cx, -1, 1), np.roll(cy, -1, 1)
    dy = ry - cy
    grid = np.arange(S, dtype=np.float64) / S

    # live[p,k,j]: edge k crosses scanline y=y_j (half-open, vertex-safe)
    up = (cy[:, :, None] <= grid) & (grid < ry[:, :, None])
    dn = (ry[:, :, None] <= grid) & (grid < cy[:, :, None])
    live = up | dn
    with np.errstate(divide="ignore", invalid="ignore"):
        frac = (grid[None, None, :] - cy[:, :, None]) / dy[:, :, None]
    XC = cx[:, :, None] + frac * (rx - cx)[:, :, None]
    XC = np.nan_to_num(np.where(live, XC, 0.0), nan=0.0, posinf=0.0, neginf=0.0)

    sgam = K_SIGN * dy                            # SIGNED tanh slope: H folds in
    SXI = sgam[:, :, None] * grid[None, None, :]  # (NCON, KV, S) over i
    SXC = sgam[:, :, None] * XC                   # (NCON, KV, S) over j
    dead_sxc = np.minimum(0.0, sgam)[:, :, None] - 50.0   # u>=50 -> T=+1.0
    SXC = np.where(live, SXC, dead_sxc)
    C = 0.5 * (~live).sum(axis=1)                 # dead-cell bias per (p, j)

    mask = np.zeros((128, CPC), np.float16)
    for lc in range(CPC):
        mask[lc * KV : (lc + 1) * KV, lc] = np.float16(0.5)

    in_maps = []
    for core in range(NCORES):
        f32 = np.zeros((128, 2 * S + CPC), np.float32)
        for lc in range(CPC):
            p = core * CPC + lc
            rows = slice(lc * KV, (lc + 1) * KV)
            f32[rows, :S] = SXI[p]
            f32[rows, S : 2 * S] = SXC[p]
            f32[:, 2 * S + lc] = C[p]             # per-j bias, partition = j
        in_maps.append({"cf32": f32, "cf16": mask})
    return in_maps


def kernel(contour, size):
    contour = np.asarray(contour, dtype=np.float32)
    size = int(size)
    assert contour.shape == (B, N, KV, 2), contour.shape
    assert size == S, size

    from concourse.bass_utils import run_bass_kernel_spmd

    nc = _get_nc()
    in_maps = _make_in_maps(contour)
    res = run_bass_kernel_spmd(nc, in_maps, core_ids=list(range(NCORES)))
    # per-core out is [j, (i, contour)]; undo the layout on host
    cores = [
        res.results[i]["out"].reshape(S, S, CPC).transpose(2, 1, 0)
        for i in range(NCORES)
    ]
    full = np.concatenate(cores, axis=0)
    return full.reshape(B, N, S, S).astype(np.float32)
